# revision 1
# baseline (speedup 1.0000x reference)
"""Trainium2 Bass kernel for per-expert 2-layer MLP (grouped GEMM -> GELU -> grouped GEMM).

reference: hidden = einsum('end,edh->enh', x, w1); gelu(erf); out = einsum('enh,ehd->end', h, w2)
shapes:    x [16, 2048, 1024] f32, w1 [16, 1024, 4096] f32, w2 [16, 4096, 1024] f32

Expert-parallel over 8 NeuronCores: core c owns experts [2c, 2c+1], no
cross-core communication.  Per core, per expert:

  phase A:  actT[h, n] = gelu(w1[d, h].T @ xT[d, n])   (PE matmul, contraction d)
  phase B:  out[n, d'] = actT[h, n].T @ w2[h, d']      (PE matmul, contraction h)

Layout trick: matmul1 with w1 as the stationary operand directly yields
hidden TRANSPOSED ([h, n]) which is exactly the lhsT layout matmul2 needs.
x is pre-transposed (and pre-cast to fp16, like the weights) on the host as
part of sharding, so every device-side DMA is a natural contiguous load and
the PE does nothing but the 4096 productive matmuls.  Matmuls run in fp16
with fp32 PSUM accumulation; GELU (erf) runs on ScalarE out of PSUM.
"""

import os
import sys

import numpy as np

for _p in ("/opt/trn_rl_repo", "/root/.axon_site/_ro/trn_rl_repo"):
    if os.path.isdir(_p) and _p not in sys.path:
        sys.path.append(_p)

import concourse.bacc as bacc
import concourse.tile as tile
from concourse import mybir
from concourse.bass_utils import run_bass_kernel_spmd

E, N, D, H = 16, 2048, 1024, 4096
NCORES = 8
EPC = E // NCORES        # experts per core                     = 2
P = 128                  # SBUF partitions
FD = 512                 # matmul moving free dim
NB = 512                 # token block per phase-A/B iteration
N_BLOCKS = N // NB       # = 4
N_SUB = NB // P          # row sub-blocks per token block       = 4
KD = D // P              # d-blocks (contraction of matmul 1)   = 8
KH = H // P              # h-blocks (contraction of matmul 2)   = 32
DC = D // FD             # d' chunks (free dim of matmul 2)     = 2
F16 = mybir.dt.float16
F32 = mybir.dt.float32

_CACHE = {}


def _build():
    nc = bacc.Bacc(None, target_bir_lowering=False)
    xt_d = nc.declare_dram_parameter("xt", [EPC, D, N], F16, isOutput=False)
    w1_d = nc.declare_dram_parameter("w1", [EPC, D, H], F16, isOutput=False)
    w2_d = nc.declare_dram_parameter("w2", [EPC, H, D], F16, isOutput=False)
    out_d = nc.declare_dram_parameter("out", [EPC, N, D], F32, isOutput=True)

    with (
        tile.TileContext(nc) as tc,
        tc.tile_pool(name="w1sb", bufs=1) as w1_pool,
        tc.tile_pool(name="w2sb", bufs=1) as w2_pool,
        tc.tile_pool(name="xT", bufs=2) as xt_pool,
        tc.tile_pool(name="actT", bufs=1) as act_pool,
        tc.tile_pool(name="osb", bufs=3) as out_pool,
        tc.tile_pool(name="ps_1", bufs=4, space="PSUM") as ps1_pool,
        tc.tile_pool(name="ps_2", bufs=4, space="PSUM") as ps2_pool,
    ):

        def emit_w1_loads(e):
            """4 batched strided DMAs, column-chunk-major: phase A's first
            h-blocks unblock after one 2MB chunk, and few triggers keep the
            HWDGE queue free (each dma_start costs ~0.6us of queue time)."""
            w1_sb = w1_pool.tile([P, KD, H], F16, tag="w1")
            w1_view = w1_d[e].rearrange("(k p) h -> p k h", p=P)
            # tiny first slice so phase A's first h-block unblocks ASAP
            bounds = [0, P, 1024, 2048, 3072, H]
            for lo, hi in zip(bounds, bounds[1:]):
                nc.scalar.dma_start(
                    out=w1_sb[:, :, lo:hi], in_=w1_view[:, :, lo:hi]
                )
            return w1_sb

        def emit_w2_loads(e):
            w2_sb = w2_pool.tile([P, KH, D], F16, tag="w2")
            w2_view = w2_d[e].rearrange("(h p) d -> p h d", p=P)
            HB = KH // 4
            for c in range(4):
                nc.scalar.dma_start(
                    out=w2_sb[:, c * HB : (c + 1) * HB, :],
                    in_=w2_view[:, c * HB : (c + 1) * HB, :],
                )
            return w2_sb

        def emit_x_loads(e, nb):
            n0 = nb * NB
            xt_sb = xt_pool.tile([P, KD, NB], F16, tag="xT")
            xt_view = xt_d[e].rearrange("(k p) n -> p k n", p=P)
            nc.sync.dma_start(out=xt_sb[:, :, :], in_=xt_view[:, :, n0 : n0 + NB])
            return xt_sb

        def emit_phase_a(w1_sb, xt_sb):
            actT = act_pool.tile([P, KH, NB], F16, tag="actT")
            for h in range(KH):
                ps1 = ps1_pool.tile([P, NB], F32, tag="ps1")
                for k in range(KD):
                    nc.tensor.matmul(
                        ps1,
                        lhsT=w1_sb[:, k, h * P : (h + 1) * P],
                        rhs=xt_sb[:, k, :],
                        start=(k == 0),
                        stop=(k == KD - 1),
                    )
                nc.scalar.activation(actT[:, h, :], ps1, mybir.ActivationFunctionType.Gelu)
            return actT

        def emit_phase_b(e, nb, actT, w2_sb):
            n0 = nb * NB
            for s in range(N_SUB):
                osb = out_pool.tile([P, D], F32, tag="osb")
                for c in range(DC):
                    ps2 = ps2_pool.tile([P, FD], F32, tag="ps2")
                    for h in range(KH):
                        nc.tensor.matmul(
                            ps2,
                            lhsT=actT[:, h, s * P : (s + 1) * P],
                            rhs=w2_sb[:, h, c * FD : (c + 1) * FD],
                            start=(h == 0),
                            stop=(h == KH - 1),
                        )
                    nc.vector.tensor_copy(osb[:, c * FD : (c + 1) * FD], ps2)
                nc.sync.dma_start(out=out_d[e, n0 + s * P : n0 + (s + 1) * P, :], in_=osb)

        w1_cur = emit_w1_loads(0)
        w1_next = None
        w2_cur = None
        for e in range(EPC):
            for nb in range(N_BLOCKS):
                xt_sb = emit_x_loads(e, nb)
                actT = emit_phase_a(w1_cur, xt_sb)
                if nb == 0:
                    if e == 0:
                        # Stall the w2 slot until phase A is underway: its 8MB
                        # stream otherwise saturates the paired-core HBM window
                        # (~680 of 716 GB/s) and starves the w1 chunk stream.
                        gate = w2_pool.tile([P, 4], F32, tag="w2")
                        nc.vector.tensor_copy(gate, actT[:, 4, 0:4])
                    w2_cur = emit_w2_loads(e)
                if nb == N_BLOCKS - 1 and e + 1 < EPC:
                    w1_next = emit_w1_loads(e + 1)
                emit_phase_b(e, nb, actT, w2_cur)
            w1_cur = w1_next

    nc.compile()
    return nc


def _get_nc():
    if "nc" not in _CACHE:
        _CACHE["nc"] = _build()
    return _CACHE["nc"]


def _run(inputs, trace=False, trace_cores=None):
    x = np.asarray(inputs["x"], dtype=np.float32).astype(np.float16)
    w1 = np.asarray(inputs["w1"], dtype=np.float32).astype(np.float16)
    w2 = np.asarray(inputs["w2"], dtype=np.float32).astype(np.float16)
    xt = np.ascontiguousarray(np.swapaxes(x, 1, 2))  # [E, D, N]
    nc = _get_nc()
    in_maps = [
        {
            "xt": xt[c * EPC : (c + 1) * EPC],
            "w1": np.ascontiguousarray(w1[c * EPC : (c + 1) * EPC]),
            "w2": np.ascontiguousarray(w2[c * EPC : (c + 1) * EPC]),
        }
        for c in range(NCORES)
    ]
    res = run_bass_kernel_spmd(
        nc, in_maps, list(range(NCORES)), trace=trace, trace_cores=trace_cores
    )
    out = np.concatenate([res.results[c]["out"] for c in range(NCORES)], axis=0)
    return out.astype(np.float32, copy=False), res


def kernel(**inputs) -> np.ndarray:
    out, _ = _run(inputs, trace=False)
    return out



# revision 2
# speedup vs baseline: 1.0019x; 1.0019x over previous
"""Trainium2 Bass kernel for per-expert 2-layer MLP (grouped GEMM -> GELU -> grouped GEMM).

reference: hidden = einsum('end,edh->enh', x, w1); gelu(erf); out = einsum('enh,ehd->end', h, w2)
shapes:    x [16, 2048, 1024] f32, w1 [16, 1024, 4096] f32, w2 [16, 4096, 1024] f32

Expert-parallel over 8 NeuronCores: core c owns experts [2c, 2c+1], no
cross-core communication.  Per core, per expert:

  phase A:  actT[h, n] = gelu(w1[d, h].T @ xT[d, n])   (PE matmul, contraction d)
  phase B:  out[n, d'] = actT[h, n].T @ w2[h, d']      (PE matmul, contraction h)

The PE roofline for this fp16 workload is 2,097,152 moving rows / 2.4 GHz
= 873.8 us per core; everything here is about keeping the PE gapless:

- All DRAM operands are pre-swizzled on the host so every DMA descriptor
  moves a >=2KB contiguous run on both the DRAM and SBUF side (the naive
  [D,N]/[D,H] layouts produce 256B-1KB packets that are descriptor-rate
  bound and stall the first matmul ~7us).
- w1 arrives in 128-h-column chunks so phase A's h-block j only waits on
  chunk j, not the full 8MB tensor.
- A run of zero-input warmup matmuls issues while the first DMAs are in
  flight: they ramp the PE out of its low DVFS p-state (634ns -> 380ns
  per 512-row matmul) before the first real chain starts.
- Phase B shares each stationary actT column-block between the two d'
  chunks (h outer, c inner) so consecutive matmuls reuse loaded weights.
Matmuls run in fp16 with fp32 PSUM accumulation; GELU (erf) runs on
ScalarE out of PSUM.
"""

import os
import sys

import numpy as np

for _p in ("/opt/trn_rl_repo", "/root/.axon_site/_ro/trn_rl_repo"):
    if os.path.isdir(_p) and _p not in sys.path:
        sys.path.append(_p)

import concourse.bacc as bacc
import concourse.tile as tile
from concourse import mybir
from concourse.bass_utils import run_bass_kernel_spmd

E, N, D, H = 16, 2048, 1024, 4096
NCORES = 8
EPC = E // NCORES        # experts per core                     = 2
P = 128                  # SBUF partitions
FD = 512                 # matmul moving free dim
NB = 512                 # token block per phase-A/B iteration
N_BLOCKS = N // NB       # = 4
N_SUB = NB // P          # row sub-blocks per token block       = 4
KD = D // P              # d-blocks (contraction of matmul 1)   = 8
KH = H // P              # h-blocks (contraction of matmul 2)   = 32
DC = D // FD             # d' chunks (free dim of matmul 2)     = 2
F16 = mybir.dt.float16
F32 = mybir.dt.float32
N_WARMUP = 48            # zero matmuls to ramp the PE p-state

_CACHE = {}


def _build():
    nc = bacc.Bacc(None, target_bir_lowering=False)
    # host-swizzled layouts (see _run): every DMA is contiguous-run friendly
    xb_d = nc.declare_dram_parameter("xb", [EPC, N_BLOCKS, P, KD, NB], F16, isOutput=False)
    w1_d = nc.declare_dram_parameter("w1b", [EPC, KH, P, KD, P], F16, isOutput=False)
    w2_d = nc.declare_dram_parameter("w2b", [EPC, P, KH, D], F16, isOutput=False)
    out_d = nc.declare_dram_parameter("out", [EPC, N, D], F32, isOutput=True)

    with (
        tile.TileContext(nc) as tc,
        tc.tile_pool(name="warm", bufs=1) as warm_pool,
        tc.tile_pool(name="w1sb", bufs=1) as w1_pool,
        tc.tile_pool(name="w2sb", bufs=1) as w2_pool,
        tc.tile_pool(name="xT", bufs=2) as xt_pool,
        tc.tile_pool(name="actT", bufs=1) as act_pool,
        tc.tile_pool(name="osb", bufs=3) as out_pool,
        tc.tile_pool(name="ps_1", bufs=4, space="PSUM") as ps1_pool,
        tc.tile_pool(name="ps_2", bufs=4, space="PSUM") as ps2_pool,
    ):

        def emit_warmup():
            """Zero matmuls with no DMA dependency: they start right after
            the prologue barrier while the first loads are still in flight
            and walk the PE up to its max p-state (~3us of busy time)."""
            wz = warm_pool.tile([P, P], F16, tag="wz")
            nc.vector.memset(wz, 0.0)
            for _ in range(N_WARMUP):
                pw = ps1_pool.tile([P, NB], F32, tag="ps1")
                nc.tensor.matmul(pw[:, 0:64], lhsT=wz, rhs=wz[:, 0:64],
                                 start=True, stop=True)

        def emit_w1_loads(e):
            """w1 for expert e: SBUF [p, hc, k, hj] (hc = 32 chunks of 128 h).
            DRAM layout [e, hc, p, k, hj] makes each chunk a 2KB-run DMA.
            First 4 chunks go as single-chunk DMAs so phase A's first
            h-blocks unblock ASAP; the rest batch 4 chunks per trigger."""
            w1_sb = w1_pool.tile([P, KH, KD, P], F16, tag="w1")
            w1_view = w1_d[e].rearrange("c p k h -> p c k h")
            for hc in range(4):
                nc.scalar.dma_start(
                    out=w1_sb[:, hc : hc + 1], in_=w1_view[:, hc : hc + 1]
                )
            for c in range(1, 8):
                nc.scalar.dma_start(
                    out=w1_sb[:, 4 * c : 4 * c + 4], in_=w1_view[:, 4 * c : 4 * c + 4]
                )
            return w1_sb

        def emit_w2_loads(e):
            """whole-expert w2: DRAM [e, p, h, d] gives 64KB runs/partition."""
            w2_sb = w2_pool.tile([P, KH, D], F16, tag="w2")
            nc.scalar.dma_start(out=w2_sb, in_=w2_d[e])
            return w2_sb

        def emit_x_loads(e, nb):
            """one 8KB-run DMA per 512-token block."""
            xt_sb = xt_pool.tile([P, KD, NB], F16, tag="xT")
            nc.sync.dma_start(out=xt_sb, in_=xb_d[e, nb])
            return xt_sb

        def emit_phase_a(w1_sb, xt_sb):
            actT = act_pool.tile([P, KH, NB], F16, tag="actT")
            for h in range(KH):
                ps1 = ps1_pool.tile([P, NB], F32, tag="ps1")
                for k in range(KD):
                    nc.tensor.matmul(
                        ps1,
                        lhsT=w1_sb[:, h, k],
                        rhs=xt_sb[:, k],
                        start=(k == 0),
                        stop=(k == KD - 1),
                    )
                nc.scalar.activation(actT[:, h], ps1, mybir.ActivationFunctionType.Gelu)
            return actT

        def emit_phase_b(e, nb, actT, w2_sb):
            n0 = nb * NB
            for s in range(N_SUB):
                osb = out_pool.tile([P, D], F32, tag="osb")
                pa = ps2_pool.tile([P, FD], F32, tag="ps2")
                pb = ps2_pool.tile([P, FD], F32, tag="ps2")
                for h in range(KH):
                    lhsT = actT[:, h, s * P : (s + 1) * P]
                    nc.tensor.matmul(pa, lhsT=lhsT, rhs=w2_sb[:, h, 0:FD],
                                     start=(h == 0), stop=(h == KH - 1))
                    nc.tensor.matmul(pb, lhsT=lhsT, rhs=w2_sb[:, h, FD:D],
                                     start=(h == 0), stop=(h == KH - 1))
                nc.vector.tensor_copy(osb[:, 0:FD], pa)
                nc.vector.tensor_copy(osb[:, FD:D], pb)
                nc.sync.dma_start(out=out_d[e, n0 + s * P : n0 + (s + 1) * P, :], in_=osb)

        emit_warmup()
        w1_cur = emit_w1_loads(0)
        w1_next = None
        w2_cur = None
        for e in range(EPC):
            for nb in range(N_BLOCKS):
                xt_sb = emit_x_loads(e, nb)
                actT = emit_phase_a(w1_cur, xt_sb)
                if nb == 0:
                    if e == 0:
                        # Stall the w2 slot until phase A is underway: its 8MB
                        # stream otherwise competes with the startup-critical
                        # w1 chunk + x block loads for the HBM window.
                        gate = w2_pool.tile([P, 4], F32, tag="w2")
                        nc.vector.tensor_copy(gate, actT[:, 4, 0:4])
                    w2_cur = emit_w2_loads(e)
                if nb == N_BLOCKS - 1 and e + 1 < EPC:
                    w1_next = emit_w1_loads(e + 1)
                emit_phase_b(e, nb, actT, w2_cur)
            w1_cur = w1_next

    nc.compile()
    return nc


def _get_nc():
    if "nc" not in _CACHE:
        _CACHE["nc"] = _build()
    return _CACHE["nc"]


def _prep(inputs):
    x = np.asarray(inputs["x"], dtype=np.float32).astype(np.float16)
    w1 = np.asarray(inputs["w1"], dtype=np.float32).astype(np.float16)
    w2 = np.asarray(inputs["w2"], dtype=np.float32).astype(np.float16)
    # xb[e, nb, p, k, nj] = x[e, nb*512+nj, k*128+p]
    xb = np.ascontiguousarray(
        x.reshape(E, N_BLOCKS, NB, KD, P).transpose(0, 1, 4, 3, 2)
    )
    # w1b[e, hc, p, k, hj] = w1[e, k*128+p, hc*128+hj]
    w1b = np.ascontiguousarray(
        w1.reshape(E, KD, P, KH, P).transpose(0, 3, 2, 1, 4)
    )
    # w2b[e, p, hb, d] = w2[e, hb*128+p, d]
    w2b = np.ascontiguousarray(
        w2.reshape(E, KH, P, D).transpose(0, 2, 1, 3)
    )
    return xb, w1b, w2b


def _run(inputs, trace=False, trace_cores=None):
    xb, w1b, w2b = _prep(inputs)
    nc = _get_nc()
    in_maps = [
        {
            "xb": xb[c * EPC : (c + 1) * EPC],
            "w1b": w1b[c * EPC : (c + 1) * EPC],
            "w2b": w2b[c * EPC : (c + 1) * EPC],
        }
        for c in range(NCORES)
    ]
    res = run_bass_kernel_spmd(
        nc, in_maps, list(range(NCORES)), trace=trace, trace_cores=trace_cores
    )
    out = np.concatenate([res.results[c]["out"] for c in range(NCORES)], axis=0)
    return out.astype(np.float32, copy=False), res


def kernel(**inputs) -> np.ndarray:
    out, _ = _run(inputs, trace=False)
    return out
